# revision 90
# baseline (speedup 1.0000x reference)
"""Distributed Trainium2 Bass kernel for AlignmentContrastiveLoss (packed).

Reference computation (B=256, L_im=37, L_s=33, D=1024):
    im  = l2norm(im_set)[:, 1:, :]   masked by im_len-1     [B, 36, D]
    s   = l2norm(s_seq)[:, 1:-2, :]  masked by s_len-3      [B, 30, D]
    align[b,c,i,j] = im[b,i] . s[c,j]   (masked entries -> 0)
    scores[b,c] = sum_j max_i align[b,c,i,j]
    loss = sum_b relu(M + max_{c!=b} scores[b,c] - scores[b,b])
         + sum_c relu(M + max_{b!=c} scores[b,c] - scores[c,c])

Sparsity exploitation (the big win vs a dense kernel): only valid im
regions / s words are ever loaded or multiplied.
  * s side: all valid (c,j) rows are packed densely (per 128-sentence
    half, zero-padded to 128-row tiles) -> NT ~ 36 instead of 60 tiles.
    Invalid words contribute exactly 0 to scores, so dropping them is
    exact; the host-built 0/1 G matrix maps packed rows -> sentences.
  * im side: images are sorted by region count and dealt round-robin
    (rank r -> core r%8, slot r//8) so all 8 cores share one compiled
    slot profile; slot lengths are the per-group max quantized to
    multiples of 4 (<= 7 distinct lengths -> few DVE reduce runs).
    Images with im_l < 36 get >= 1 zero pad row in their slot, which
    reproduces the reference's max-with-0 semantics exactly.
  * per-core matmul: s packed rows stationary (bf16, host-cast),
    normalized im rows moving; max-over-i from PSUM on DVE; 1/|s| folded
    in post-max on ACT; 0/1 G matmuls accumulate scoresT [128 x 32] per
    half; s norms are computed on-device from a sharded row slice and
    AllGathered; final per-core stats AllGathered (768 floats) and the
    scalar loss computed redundantly on every core.
"""

import math
import os
import sys

import numpy as np

for _p in ("/opt/trn_rl_repo", "/root/.axon_site/_ro/trn_rl_repo"):
    if os.path.isdir(_p) and _p not in sys.path:
        sys.path.append(_p)

import ml_dtypes

import concourse.bass as bass
import concourse.mybir as mybir
import concourse.tile as tile
from concourse import bacc
from concourse.bass_utils import run_bass_kernel_spmd


def _act_raw(eng, out, in_, func, bias=0.0, scale=1.0):
    """Emit an InstActivation directly (nc.scalar.activation refuses
    Reciprocal/Rsqrt on accuracy grounds; our tolerance is ~2e-2, so the
    table approximation is more than fine here)."""
    ins = [eng.lower_ap(in_)]
    b = eng.bass.const_aps.scalar_like(bias, in_) if isinstance(bias, float) else bias
    for arg in (b, scale, 0.0):
        if isinstance(arg, (int, float)):
            ins.append(mybir.ImmediateValue(dtype=mybir.dt.float32, value=float(arg)))
        else:
            ins.append(eng.lower_ap(arg))
    return eng.add_instruction(
        mybir.InstActivation(
            name=eng.bass.get_next_instruction_name(),
            func=func, ins=ins, outs=[eng.lower_ap(out)],
        )
    )


def _ensure_axon_hooks():
    """Some agent images ship an ``antenv`` without ``axon_hooks``, but
    bass_utils hard-imports it when trace=True.  Provide the registry and,
    when libaxon_pjrt.so is available, the real NTFF profile hook."""
    import types

    try:
        import antenv.axon_hooks  # noqa: F401
        return
    except ImportError:
        pass
    try:
        import antenv
    except ImportError:
        return
    mod = types.ModuleType("antenv.axon_hooks")
    mod._hook = None
    mod.set_axon_ntff_profile_hook = lambda h: setattr(mod, "_hook", h)
    mod.get_axon_ntff_profile_hook = lambda: mod._hook
    sys.modules["antenv.axon_hooks"] = mod
    antenv.axon_hooks = mod
    so_path = "/opt/axon/libaxon_pjrt.so"
    try:
        import trn_agent_boot.trn_boot as _tb
        if os.path.exists(so_path):
            mod._hook = _tb._ntff_profile_via_ctypes(so_path)
    except Exception:
        pass


_ensure_axon_hooks()

F32 = mybir.dt.float32
F32R = mybir.dt.float32r
BF16 = mybir.dt.bfloat16
F8E4 = mybir.dt.float8e4
I32 = mybir.dt.int32
AX = mybir.AxisListType
ALU = mybir.AluOpType
ACT = mybir.ActivationFunctionType
BF = ml_dtypes.bfloat16
F8 = ml_dtypes.float8_e4m3

# fp8 alignment matmuls (DoubleRow: 2 contraction tiles / instruction).
# im rows scaled x128, s rows x32 pre-quantization; 1/4096 folded into the
# post-max 1/|s| scale, so downstream math is unchanged.
MM_F8 = os.environ.get("KF8", "1") == "1"
IM_UPS = 128.0
S_UPS = 32.0

NCORES = 8
B, LI, LS, D = 256, 36, 30, 1024
KC = D // 128               # 8 contraction chunks
MARGIN, EPS, NEG = 0.2, 1e-12, -1.0e9

DRAIN_LAG = 2               # G-matmul for tile t emitted at loop step t+2

LAST_RESULT = None  # BassKernelResults of the most recent run (for test harness)
DEBUG = os.environ.get("KDBG", "0") == "1"
DBG_T = int(os.environ.get("KDBG_T", "0"))   # which tile's mx to dump


# ---------------------------------------------------------------------------
# layout plan (depends only on im_len / s_len)
# ---------------------------------------------------------------------------

def make_plan(im_len, s_len):
    im_l = (np.asarray(im_len).astype(np.int64) - 1)    # 9..36 valid regions
    s_l = (np.asarray(s_len).astype(np.int64) - 3)      # 5..30 valid words
    # image slots: sort desc, deal rank-groups of 8 across cores
    order = np.argsort(-im_l, kind="stable")
    assign = order.reshape(32, NCORES)                  # [slot, core] -> b
    gmax = im_l[assign].max(axis=1)
    # quantize to mult of 4; strictly > im_l when im_l < LI (the zero-pad
    # row in-slot reproduces the reference max-with-0)
    slot_len = np.where(gmax == LI, LI, np.minimum(LI, 4 * ((gmax + 4) // 4)))
    slot_off = np.concatenate([[0], np.cumsum(slot_len)])
    SL = int(slot_off[-1])
    BIc = ((SL + 127) // 128) * 128
    NRT = BIc // 128
    # chunks: greedy pack slots into <=512-col PSUM banks, split at slots
    bounds = []
    cur_start = 0
    s0 = 0
    for r in range(32):
        if slot_off[r + 1] - cur_start > 512:
            bounds.append((cur_start, s0, r))
            cur_start = int(slot_off[r])
            s0 = r
    bounds.append((cur_start, s0, 32))
    chunks = []
    for noff, cs, se in bounds:
        runs = []
        r = cs
        while r < se:
            L = int(slot_len[r])
            cnt = 1
            while r + cnt < se and slot_len[r + cnt] == L:
                cnt += 1
            runs.append((int(slot_off[r]), L, cnt, r))
            r += cnt
        chunks.append((noff, int(slot_off[se] - noff), runs))
    # sentence packing: per half, all valid (c,j) rows then pad to 128
    cj_rows = []
    half_nt = []
    for h in range(2):
        for c in range(128 * h, 128 * h + 128):
            for j in range(int(s_l[c])):
                cj_rows.append((c, 1 + j))
        while len(cj_rows) % 128:
            cj_rows.append(None)
        half_nt.append(len(cj_rows) // 128)
    NT0 = half_nt[0]
    NT = half_nt[1]
    sig = (NT0, NT, SL, BIc, MM_F8, tuple(int(x) for x in slot_len))
    return dict(im_l=im_l, s_l=s_l, assign=assign, slot_len=slot_len,
                slot_off=slot_off, SL=SL, BIc=BIc, NRT=NRT, chunks=chunks,
                cj_rows=cj_rows, NT0=NT0, NT=NT, sig=sig)


# ---------------------------------------------------------------------------
# device program
# ---------------------------------------------------------------------------

def build_nc(plan):
    NT, NT0 = plan["NT"], plan["NT0"]
    NRT, BIc, SL = plan["NRT"], plan["BIc"], plan["SL"]
    chunks = plan["chunks"]

    nc = bacc.Bacc(None, target_bir_lowering=False, debug=False, num_devices=NCORES)

    MMDT = F8E4 if MM_F8 else BF16
    imr_e = nc.declare_dram_parameter("imr", [BIc, D], BF16, isOutput=False)
    srow_e = nc.declare_dram_parameter("srow", [NT, 128, D], BF16, isOutput=False)
    st_e = nc.declare_dram_parameter("st", [NT, 128, KC, 128], MMDT, isOutput=False)
    ident_e = nc.declare_dram_parameter("ident", [128, 128], F32, isOutput=False)
    gmat_e = nc.declare_dram_parameter("gmat", [128, NT * 128], BF16, isOutput=False)
    pos0_e = nc.declare_dram_parameter("pos0", [128, 32], F32, isOutput=False)
    pos1_e = nc.declare_dram_parameter("pos1", [128, 32], F32, isOutput=False)
    post0_e = nc.declare_dram_parameter("post0", [32, 128], F32, isOutput=False)
    post1_e = nc.declare_dram_parameter("post1", [32, 128], F32, isOutput=False)
    out_e = nc.declare_dram_parameter("out", [1, 1], F32, isOutput=True)
    if DEBUG:
        dbgsc_e = nc.declare_dram_parameter("dbgsc", [128, NT], F32, isOutput=True)
        dbgmx_e = nc.declare_dram_parameter("dbgmx", [128, 32], F32, isOutput=True)
        dbgS_e = nc.declare_dram_parameter("dbgS", [128, 64], F32, isOutput=True)
        dbgpay_e = nc.declare_dram_parameter("dbgpay", [128, 6], F32, isOutput=True)

    with tile.TileContext(nc) as tc:
        from contextlib import ExitStack

        with ExitStack() as ctx:
            dram = ctx.enter_context(tc.tile_pool(name="dram", bufs=1, space="DRAM"))
            const = ctx.enter_context(tc.tile_pool(name="const", bufs=1))
            small = ctx.enter_context(tc.tile_pool(name="small", bufs=1))
            stp = ctx.enter_context(tc.tile_pool(name="stp", bufs=10))
            mxp = ctx.enter_context(tc.tile_pool(name="mxp", bufs=1))
            prep = ctx.enter_context(tc.tile_pool(name="prep", bufs=3))
            # PSUM budget (8 banks): align ring 7 + S accumulator 1; the
            # epilogue scratch tiles borrow the align ring
            pal = ctx.enter_context(tc.tile_pool(name="pal", bufs=7, space="PSUM"))

            # DRAM scratch
            sync1_d = dram.tile([1, 1], F32, tag="sync1_d")
            sync8_d = dram.tile([NCORES, 1], F32, tag="sync8_d")
            pay_d = dram.tile([128, 6], F32, tag="pay_d")
            ag2_d = dram.tile([NCORES * 128, 6], F32, tag="ag2_d")

            def epi_psum(shape, name):
                return pal.tile(shape, F32, tag="al", name=name)

            ident = const.tile([128, 128], F32, tag="ident")
            nc.sync.dma_start(out=ident[:, :], in_=ident_e[:, :])

            # barrier collective: absorbs core launch skew early (hidden
            # under the head) so the payload AllGather's peer wait is short
            zz = small.tile([1, 1], F32, tag="zz")
            nc.gpsimd.memset(zz[:, :], 0.0)
            nc.sync.dma_start(out=sync1_d[:, :], in_=zz[:, :])
            nc.gpsimd.collective_compute(
                "AllGather", ALU.bypass,
                replica_groups=[list(range(NCORES))],
                ins=[sync1_d.opt()],
                outs=[sync8_d.opt()],
            )

            # ---- phase 1: im prep, one fused chain per row-tile ----
            ident_bf = const.tile([128, 128], BF16, tag="ident_bf")
            nc.scalar.copy(ident_bf[:, :], ident[:, :])
            epsb = const.tile([128, 1], F32, tag="epsb")
            nc.gpsimd.memset(epsb[:, :], EPS * EPS)
            # prewarm the Rsqrt ACT table before the squares occupy ACT
            tblw = small.tile([128, 1], F32, tag="tblw")
            _act_raw(nc.scalar, tblw[:, :], epsb[:, :], ACT.Rsqrt)

            imr_tiles = []
            for rt in range(NRT):
                imr_t = prep.tile([128, D], BF16, tag="imld", name="imr_t", bufs=NRT)
                nc.sync.dma_start(out=imr_t[:, :], in_=imr_e[128 * rt:128 * (rt + 1), :])
                imr_tiles.append(imr_t)

            # normalize+transpose fused on PE: the "ident" of the transpose
            # matmul is diag(IM_UPS/|row|), so raw bf16 tiles feed PE as soon
            # as their tiny rcp vector is ready; fp8 cast on the PSUM->imt
            # copy.  Squares alternate ACT/DVE so the two serial chains run
            # in parallel from the moment each imr DMA lands.
            imt = const.tile([128, KC * BIc], MMDT, tag="imt")
            nrm_scale = 1.0 / (IM_UPS * IM_UPS) if MM_F8 else 1.0
            for rt in range(NRT):
                imr_t = imr_tiles[rt]
                ssq = small.tile([128, 1], F32, tag=f"imssq{rt}", name="ssq")
                if rt % 2 == 0:
                    sq = prep.tile([128, D], F32, tag="imsq", name="sq")
                    nc.scalar.activation(sq[:, :], imr_t[:, :], ACT.Square,
                                         accum_out=ssq[:, :])
                else:
                    sq = prep.tile([128, D], F32, tag="imsqv", name="sq")
                    nc.vector.scalar_tensor_tensor(
                        out=sq[:, :], in0=imr_t[:, :], scalar=1.0,
                        in1=imr_t[:, :], op0=ALU.mult, op1=ALU.mult,
                        accum_out=ssq[:, :],
                    )
                rcp = small.tile([128, 1], F32, tag=f"imrcp{rt}")
                _act_raw(nc.scalar, rcp[:, :], ssq[:, :], ACT.Rsqrt,
                         bias=epsb[:, :], scale=nrm_scale)
                diag_rcp = small.tile([128, 128], BF16, tag=f"diagr{rt}")
                nc.scalar.mul(diag_rcp[:, :], ident_bf[:, :], mul=rcp[:, :])
                for k in range(KC):
                    # packed [128,128] weight tile (partition pitch == width;
                    # slices of the wide imr tile are LDW-opt-incompatible)
                    imw = prep.tile([128, 128], BF16, tag="imw", name="imw",
                                    bufs=16)
                    nc.sync.dma_start(
                        out=imw[:, :],
                        in_=imr_e[128 * rt:128 * (rt + 1),
                                  128 * k:128 * (k + 1)],
                    )
                    pst = pal.tile([128, 128], F32, tag="al", name="pst")
                    nc.tensor.matmul(pst[:, :], lhsT=imw[:, :],
                                     rhs=diag_rcp[:, :], start=True, stop=True)
                    dst = imt[:, BIc * k + 128 * rt:BIc * k + 128 * (rt + 1)]
                    if k % 2 == 0:
                        nc.vector.tensor_copy(dst, pst[:, :])
                    else:
                        nc.scalar.copy(dst, pst[:, :])

            # ---- late consts (needed by G-matmuls / epilogue) ----
            # G as NT packed [128,128] weight tiles (LDW-opt needs pitch==width)
            gtiles = []
            for t in range(NT):
                gw = const.tile([128, 128], BF16, tag=f"g{t}")
                nc.sync.dma_start(out=gw[:, :],
                                  in_=gmat_e[:, 128 * t:128 * (t + 1)])
                gtiles.append(gw)
            pos0 = const.tile([128, 32], F32, tag="pos0")
            nc.sync.dma_start(out=pos0[:, :], in_=pos0_e[:, :])
            pos1 = const.tile([128, 32], F32, tag="pos1")
            nc.sync.dma_start(out=pos1[:, :], in_=pos1_e[:, :])
            post0 = const.tile([32, 128], F32, tag="post0")
            nc.sync.dma_start(out=post0[:, :], in_=post0_e[:, :])
            post1 = const.tile([32, 128], F32, tag="post1")
            nc.sync.dma_start(out=post1[:, :], in_=post1_e[:, :])
            margin128 = const.tile([128, 1], F32, tag="margin128")
            nc.gpsimd.memset(margin128[:, :], MARGIN)

            # ---- main loop over NT packed M-tiles ----
            # S halves share one PSUM bank: cols [0:32] half0, [32:64] half1
            psacc = ctx.enter_context(tc.tile_pool(name="psacc", bufs=1, space="PSUM"))
            s_acc = psacc.tile([128, 64], F32, tag="S", name="S")
            s_ps = [s_acc[:, 0:32], s_acc[:, 32:64]]

            posm = [pos0, pos1]
            payload = small.tile([128, 6], F32, tag="payload")
            snd = [small.tile([128, 32], F32, tag=f"snd{h}", name=f"snd{h}")
                   for h in range(2)]
            trash = small.tile([128, 32], F32, tag="trash")
            negm = [small.tile([128, 32], F32, tag=f"negm{h}", name=f"negm{h}")
                    for h in range(2)]
            nc.vector.tensor_scalar_mul(negm[0][:, :], pos0[:, :], NEG)
            nc.vector.tensor_scalar_mul(negm[1][:, :], pos1[:, :], NEG)
            posr = [small.tile([128, 32], F32R, tag=f"posr{h}", name=f"posr{h}")
                    for h in range(2)]
            nc.scalar.copy(posr[0][:, :], pos0[:, :])
            nc.scalar.copy(posr[1][:, :], pos1[:, :])
            postr = [small.tile([32, 128], F32R, tag=f"postr{h}", name=f"postr{h}")
                     for h in range(2)]
            nc.scalar.copy(postr[0][:, :], post0[:, :])
            nc.scalar.copy(postr[1][:, :], post1[:, :])
            rm = small.tile([32, 2], F32, tag="rm")

            def emit_stats_h(h):
                # diag extraction: accum_out = sum(S * pos) -> payload col 2+h
                nc.vector.scalar_tensor_tensor(
                    out=trash[:, :], in0=s_ps[h], scalar=1.0, in1=posm[h][:, :],
                    op0=ALU.mult, op1=ALU.mult, accum_out=payload[:, 2 + h:3 + h],
                )
                nc.vector.tensor_add(snd[h][:, :], s_ps[h], negm[h][:, :])
                nc.vector.tensor_reduce(out=payload[:, h:h + 1], in_=snd[h][:, :],
                                        axis=AX.X, op=ALU.max)
                stp_ps = epi_psum([32, 128], "stp_ps")
                nc.tensor.transpose(stp_ps[:, :], snd[h][:, :], ident[:, :])
                nc.vector.tensor_reduce(out=rm[:, h:h + 1], in_=stp_ps[:, :],
                                        axis=AX.X, op=ALU.max)

            ssqall = small.tile([128, NT], F32, tag="ssqall")
            sscale = small.tile([128, NT], F32, tag="sscale")

            # mx_r scaling (DVE) is decoupled from the G-matmul emission: the
            # scale for tile t is queued right after t's own max-reduces, so
            # it is ready one tile before PE reaches G(t)
            pending = {}
            pending_r = {}
            next_mul = [0]
            next_g = [0]

            def drain_muls(upto):
                while next_mul[0] <= upto:
                    u = next_mul[0]
                    mx_r = mxp.tile([128, 32], BF16, tag="mx_r", name="mx_r",
                                    bufs=12)
                    nc.vector.tensor_scalar(
                        out=mx_r[:, :], in0=pending[u][:, :],
                        scalar1=sscale[:, u:u + 1], scalar2=None, op0=ALU.mult,
                    )
                    pending_r[u] = mx_r
                    next_mul[0] += 1

            def drain_g(upto):
                while next_g[0] <= upto:
                    u = next_g[0]
                    h = 0 if u < NT0 else 1
                    nc.tensor.matmul(
                        s_ps[h],
                        lhsT=gtiles[u][:, :],
                        rhs=pending_r.pop(u)[:, :],
                        start=(u == 0 or u == NT0),
                        stop=(u == NT0 - 1 or u == NT - 1),
                    )
                    next_g[0] += 1

            GSC = 6                 # sscale sqrt/rcp group size
            wm = [-1]               # highest tile with sscale emitted
            stats0 = [False]
            imt3 = imt.rearrange("p (k n) -> p k n", k=KC)
            for t in range(NT):
                srow_t = stp.tile([128, D], BF16, tag="srow", bufs=6)
                nc.sync.dma_start(out=srow_t[:, :], in_=srow_e[t, :, :])

                psc = [pal.tile([128, nw], F32, tag="al", name="ps")
                       for (_, nw, _) in chunks]
                mx = mxp.tile([128, 32], F32, tag="mx", name="mx", bufs=NT)
                if MM_F8:
                    # DoubleRow: 2 contraction k-tiles per instruction;
                    # chunk-inner keeps consecutive instrs on separate banks.
                    # One packed [128, 2*128] weight tile per k-pair (LDW-opt
                    # needs partition pitch == weight width).
                    stws = []
                    for kp in range(KC // 2):
                        stw = stp.tile([128, 256], MMDT, tag="stw", bufs=16)
                        nc.sync.dma_start(
                            out=stw.rearrange("p (k c) -> p k c", k=2),
                            in_=st_e[t, :, 2 * kp:2 * kp + 2, :],
                        )
                        stws.append(stw)
                    for kp in range(KC // 2):
                        stw3 = stws[kp].rearrange("p (k c) -> p k c", k=2)
                        for ci, (noff, nw, _) in enumerate(chunks):
                            nc.tensor.matmul(
                                psc[ci][:, :],
                                lhsT=stw3[:, :, :],
                                rhs=imt3[:, 2 * kp:2 * kp + 2, noff:noff + nw],
                                start=(kp == 0), stop=(kp == KC // 2 - 1),
                                perf_mode=mybir.MatmulPerfMode.DoubleRow,
                            )
                else:
                    st_t = stp.tile([128, KC * 128], MMDT, tag="st")
                    nc.sync.dma_start(
                        out=st_t.rearrange("p (k c) -> p k c", k=KC),
                        in_=st_e[t, :, :, :],
                    )
                    # k-outer: consecutive matmuls alternate PSUM banks,
                    # hiding the accumulator read-modify-write turnaround
                    for k in range(KC):
                        for ci, (noff, nw, _) in enumerate(chunks):
                            nc.tensor.matmul(
                                psc[ci][:, :],
                                lhsT=st_t[:, 128 * k:128 * (k + 1)],
                                rhs=imt[:, BIc * k + noff:BIc * k + noff + nw],
                                start=(k == 0), stop=(k == KC - 1),
                            )
                for ci, (noff, nw, runs) in enumerate(chunks):
                    for (off, L, cnt, slot0) in runs:
                        nc.vector.tensor_reduce(
                            out=mx[:, slot0:slot0 + cnt],
                            in_=psc[ci][:, off - noff:off - noff + cnt * L]
                            .rearrange("p (g i) -> p g i", i=L),
                            axis=AX.X, op=ALU.max,
                        )
                # per-tile s row norms (redundant per core; no collective) on
                # ACT, which is otherwise idle mid-loop; emitted after the
                # reduces so early-loop queues aren't blocked on srow DMAs
                strash = stp.tile([128, D], F32, tag="strash2", bufs=2)
                nc.scalar.activation(strash[:, :], srow_t[:, :], ACT.Square,
                                     accum_out=ssqall[:, t:t + 1])
                pending[t] = mx
                if DEBUG and t == DBG_T:
                    nc.sync.dma_start(out=dbgmx_e[:, :], in_=mx[:, :])

                if (t + 1) % GSC == 0 or t == NT - 1:
                    lo = wm[0] + 1
                    # sscale = 1/(IM_UPS*S_UPS*|s|) in one ACT op (undoes the
                    # fp8 upscales); bias keeps zero pad rows finite
                    _act_raw(nc.scalar, sscale[:, lo:t + 1], ssqall[:, lo:t + 1],
                             ACT.Rsqrt, bias=epsb[:, :],
                             scale=(IM_UPS * S_UPS) ** 2 if MM_F8 else 1.0)
                    wm[0] = t
                drain_muls(min(t, wm[0]))
                drain_g(min(t - DRAIN_LAG, next_mul[0] - 1))
                if not stats0[0] and next_g[0] > NT0 - 1 and t >= NT0 + 1:
                    emit_stats_h(0)
                    stats0[0] = True
            drain_muls(NT - 1)
            drain_g(NT - 1)
            if DEBUG:
                nc.sync.dma_start(out=dbgsc_e[:, :], in_=sscale[:, :])

            # ---- epilogue: half-1 stats + AllGather + final ----
            emit_stats_h(1)
            if DEBUG:
                sdump = small.tile([128, 64], F32, tag="sdump")
                nc.vector.tensor_copy(sdump[:, 0:32], s_ps[0])
                nc.vector.tensor_copy(sdump[:, 32:64], s_ps[1])
                nc.sync.dma_start(out=dbgS_e[:, :], in_=sdump[:, :])
            # diag per slot: pair pay col 2 with pos0 and col 3 with pos1 in
            # SEPARATE rhs columns (a core owns images of both halves, so a
            # shared 2-col rhs would contaminate: diag0[g] + diag1[g])
            dcol4 = small.tile([128, 4], F32R, tag="dcol4")
            nc.scalar.copy(dcol4[:, 0:1], payload[:, 2:3])
            nc.scalar.mul(dcol4[:, 1:2], payload[:, 2:3], mul=0.0)
            nc.scalar.copy(dcol4[:, 2:3], payload[:, 3:4])
            nc.scalar.mul(dcol4[:, 3:4], payload[:, 3:4], mul=0.0)
            rowmax = small.tile([32, 1], F32, tag="rowmax")
            nc.vector.tensor_max(rowmax[:, :], rm[:, 0:1], rm[:, 1:2])
            dfree_ps = epi_psum([32, 2], "dfree_ps")
            nc.tensor.matmul(dfree_ps[:, :], lhsT=posr[0][:, :],
                             rhs=dcol4[:, 0:2], start=True, stop=False)
            nc.tensor.matmul(dfree_ps[:, :], lhsT=posr[1][:, :],
                             rhs=dcol4[:, 2:4], start=False, stop=True)
            dfree_sb = small.tile([32, 1], F32, tag="dfree_sb")
            nc.scalar.copy(dfree_sb[:, :], dfree_ps[:, 0:1])
            rh_pre = small.tile([32, 2], F32, tag="rh_pre")
            nc.gpsimd.memset(rh_pre[:, :], 0.0)
            nc.vector.tensor_sub(rh_pre[:, 0:1], rowmax[:, :], dfree_sb[:, :])
            rowhinge = small.tile([32, 2], F32R, tag="rowhinge")
            nc.scalar.activation(rowhinge[:, :], rh_pre[:, :], ACT.Relu,
                                 bias=margin128[0:32, :])
            for h in range(2):
                rh_ps = epi_psum([128, 2], "rh_ps")
                nc.tensor.matmul(rh_ps[:, :], lhsT=postr[h][:, :],
                                 rhs=rowhinge[:, :], start=True, stop=True)
                nc.scalar.copy(payload[:, 4 + h:5 + h], rh_ps[:, 0:1])

            # payload -> DRAM (one DMA, contiguous per partition) -> AllGather
            if DEBUG:
                nc.sync.dma_start(out=dbgpay_e[:, :], in_=payload[:, :])
            nc.sync.dma_start(out=pay_d[:, :], in_=payload[:, :])
            nc.gpsimd.collective_compute(
                "AllGather", ALU.bypass,
                replica_groups=[list(range(NCORES))],
                ins=[pay_d.opt()],
                outs=[ag2_d.opt()],
            )

            # final combine (identical on every core): strided reload puts
            # sentences on partitions, (stat, core) along free; DVE reduces
            agg_in = small.tile([128, 6 * NCORES], F32, tag="agg_in")
            nc.sync.dma_start(
                out=agg_in.rearrange("p (c m) -> p c m", m=NCORES),
                in_=ag2_d.rearrange("(m p) c -> p c m", m=NCORES),
            )
            agv = agg_in.rearrange("p (c m) -> p c m", m=NCORES)
            agg = small.tile([128, 6], F32, tag="agg")
            nc.vector.tensor_reduce(out=agg[:, 0:2], in_=agv[:, 0:2, :],
                                    axis=AX.X, op=ALU.max)
            nc.vector.tensor_reduce(out=agg[:, 2:6], in_=agv[:, 2:6, :],
                                    axis=AX.X, op=ALU.add)
            ch2 = small.tile([128, 2], F32, tag="ch2")
            nc.vector.tensor_sub(ch2[:, :], agg[:, 0:2], agg[:, 2:4])
            hing4 = small.tile([128, 4], F32, tag="hing4")
            nc.scalar.activation(hing4[:, 0:2], ch2[:, :], ACT.Relu,
                                 bias=margin128[:, :])
            nc.scalar.copy(hing4[:, 2:4], agg[:, 4:6])
            psum128 = small.tile([128, 1], F32, tag="psum128")
            nc.vector.tensor_reduce(out=psum128[:, :], in_=hing4[:, :],
                                    axis=AX.X, op=ALU.add)
            lsum_ps = epi_psum([1, 128], "lsum_ps")
            nc.tensor.transpose(lsum_ps[:, :], psum128[:, :], ident[:, :])
            loss = small.tile([1, 1], F32, tag="loss")
            nc.vector.tensor_reduce(out=loss[:, :], in_=lsum_ps[:, :], axis=AX.X,
                                    op=ALU.add)
            nc.sync.dma_start(out=out_e[:, :], in_=loss[:, :])

    nc.finalize()
    return nc


# ---------------------------------------------------------------------------
# host side
# ---------------------------------------------------------------------------

def build_in_maps(plan, im_set, s_seq):
    im_set = np.asarray(im_set, dtype=np.float32)
    s_seq = np.asarray(s_seq, dtype=np.float32)
    NT, NT0, BIc = plan["NT"], plan["NT0"], plan["BIc"]
    cj = plan["cj_rows"]
    CJc = NT * 128

    s_rows = np.zeros((CJc, D), np.float32)
    idx = [i for i, row in enumerate(cj) if row is not None]
    cs = np.array([cj[i][0] for i in idx])
    js = np.array([cj[i][1] for i in idx])
    s_rows[idx] = s_seq[cs, js]
    st_f32 = np.ascontiguousarray(
        s_rows.reshape(NT, 128, KC, 128).transpose(0, 3, 2, 1))
    if MM_F8:
        st = np.clip(st_f32 * S_UPS, -240.0, 240.0).astype(F8)
    else:
        st = st_f32.astype(BF)
    srow = s_rows.reshape(NT, 128, D).astype(BF)

    gmat = np.zeros((128, NT * 128), BF)
    for t in range(NT):
        h = 0 if t < NT0 else 1
        for p in range(128):
            row = cj[128 * t + p]
            if row is not None:
                gmat[p, 128 * t + (row[0] - 128 * h)] = 1.0
    ident = np.eye(128, dtype=np.float32)

    in_maps = []
    for m in range(NCORES):
        imr = np.zeros((BIc, D), BF)
        pos0 = np.zeros((128, 32), np.float32)
        pos1 = np.zeros((128, 32), np.float32)
        for r in range(32):
            g = int(plan["assign"][r, m])
            l = int(plan["im_l"][g])
            o = int(plan["slot_off"][r])
            imr[o:o + l] = im_set[g, 1:1 + l]
            (pos0 if g < 128 else pos1)[g % 128, r] = 1.0
        in_maps.append({
            "imr": imr,
            "srow": srow,
            "st": st,
            "ident": ident,
            "gmat": gmat,
            "pos0": pos0,
            "pos1": pos1,
            "post0": np.ascontiguousarray(pos0.T),
            "post1": np.ascontiguousarray(pos1.T),
        })
    return in_maps


_NC_CACHE = {}


def kernel(im_set, s_seq, im_len, s_len):
    global LAST_RESULT
    plan = make_plan(im_len, s_len)
    key = plan["sig"]
    nc = _NC_CACHE.get(key)
    in_maps = build_in_maps(plan, im_set, s_seq)
    if nc is None:
        nc = build_nc(plan)
        _NC_CACHE[key] = nc
        # first executions of a fresh NEFF carry upload/launch-skew cost
        # (cores desynchronize by >100us); warm it up
        for _ in range(3):
            run_bass_kernel_spmd(nc, in_maps, core_ids=list(range(NCORES)))
    res = run_bass_kernel_spmd(nc, in_maps, core_ids=list(range(NCORES)))
    LAST_RESULT = res
    out = np.asarray(res.results[0]["out"], dtype=np.float32).reshape(())
    return out


# revision 94
# speedup vs baseline: 2.0948x; 2.0948x over previous
"""Distributed Trainium2 Bass kernel for AlignmentContrastiveLoss (packed).

Reference computation (B=256, L_im=37, L_s=33, D=1024):
    im  = l2norm(im_set)[:, 1:, :]   masked by im_len-1     [B, 36, D]
    s   = l2norm(s_seq)[:, 1:-2, :]  masked by s_len-3      [B, 30, D]
    align[b,c,i,j] = im[b,i] . s[c,j]   (masked entries -> 0)
    scores[b,c] = sum_j max_i align[b,c,i,j]
    loss = sum_b relu(M + max_{c!=b} scores[b,c] - scores[b,b])
         + sum_c relu(M + max_{b!=c} scores[b,c] - scores[c,c])

Sparsity exploitation (the big win vs a dense kernel): only valid im
regions / s words are ever loaded or multiplied.
  * s side: all valid (c,j) rows are packed densely (per 128-sentence
    half, zero-padded to 128-row tiles) -> NT ~ 36 instead of 60 tiles.
    Invalid words contribute exactly 0 to scores, so dropping them is
    exact; the host-built 0/1 G matrix maps packed rows -> sentences.
  * im side: images are sorted by region count and dealt round-robin
    (rank r -> core r%8, slot r//8) so all 8 cores share one compiled
    slot profile; slot lengths are the per-group max quantized to
    multiples of 4 (<= 7 distinct lengths -> few DVE reduce runs).
    Images with im_l < 36 get >= 1 zero pad row in their slot, which
    reproduces the reference's max-with-0 semantics exactly.
  * per-core matmul: s packed rows stationary (bf16, host-cast),
    normalized im rows moving; max-over-i from PSUM on DVE; 1/|s| folded
    in post-max on ACT; 0/1 G matmuls accumulate scoresT [128 x 32] per
    half; s norms are computed on-device from a sharded row slice and
    AllGathered; final per-core stats AllGathered (768 floats) and the
    scalar loss computed redundantly on every core.
"""

import math
import os
import sys

import numpy as np

for _p in ("/opt/trn_rl_repo", "/root/.axon_site/_ro/trn_rl_repo"):
    if os.path.isdir(_p) and _p not in sys.path:
        sys.path.append(_p)

import ml_dtypes

import concourse.bass as bass
import concourse.mybir as mybir
import concourse.tile as tile
from concourse import bacc
from concourse.bass_utils import run_bass_kernel_spmd


def _act_raw(eng, out, in_, func, bias=0.0, scale=1.0):
    """Emit an InstActivation directly (nc.scalar.activation refuses
    Reciprocal/Rsqrt on accuracy grounds; our tolerance is ~2e-2, so the
    table approximation is more than fine here)."""
    ins = [eng.lower_ap(in_)]
    b = eng.bass.const_aps.scalar_like(bias, in_) if isinstance(bias, float) else bias
    for arg in (b, scale, 0.0):
        if isinstance(arg, (int, float)):
            ins.append(mybir.ImmediateValue(dtype=mybir.dt.float32, value=float(arg)))
        else:
            ins.append(eng.lower_ap(arg))
    return eng.add_instruction(
        mybir.InstActivation(
            name=eng.bass.get_next_instruction_name(),
            func=func, ins=ins, outs=[eng.lower_ap(out)],
        )
    )


def _ensure_axon_hooks():
    """Some agent images ship an ``antenv`` without ``axon_hooks``, but
    bass_utils hard-imports it when trace=True.  Provide the registry and,
    when libaxon_pjrt.so is available, the real NTFF profile hook."""
    import types

    try:
        import antenv.axon_hooks  # noqa: F401
        return
    except ImportError:
        pass
    try:
        import antenv
    except ImportError:
        return
    mod = types.ModuleType("antenv.axon_hooks")
    mod._hook = None
    mod.set_axon_ntff_profile_hook = lambda h: setattr(mod, "_hook", h)
    mod.get_axon_ntff_profile_hook = lambda: mod._hook
    sys.modules["antenv.axon_hooks"] = mod
    antenv.axon_hooks = mod
    so_path = "/opt/axon/libaxon_pjrt.so"
    try:
        import trn_agent_boot.trn_boot as _tb
        if os.path.exists(so_path):
            mod._hook = _tb._ntff_profile_via_ctypes(so_path)
    except Exception:
        pass


_ensure_axon_hooks()

F32 = mybir.dt.float32
F32R = mybir.dt.float32r
BF16 = mybir.dt.bfloat16
F8E4 = mybir.dt.float8e4
I32 = mybir.dt.int32
AX = mybir.AxisListType
ALU = mybir.AluOpType
ACT = mybir.ActivationFunctionType
BF = ml_dtypes.bfloat16
F8 = ml_dtypes.float8_e4m3

# fp8 alignment matmuls (DoubleRow: 2 contraction tiles / instruction).
# im rows scaled x128, s rows x32 pre-quantization; 1/4096 folded into the
# post-max 1/|s| scale, so downstream math is unchanged.
MM_F8 = os.environ.get("KF8", "1") == "1"
IM_UPS = 128.0
S_UPS = 32.0

NCORES = 8
B, LI, LS, D = 256, 36, 30, 1024
KC = D // 128               # 8 contraction chunks
MARGIN, EPS, NEG = 0.2, 1e-12, -1.0e9

DRAIN_LAG = 2               # G-matmul for tile t emitted at loop step t+2

LAST_RESULT = None  # BassKernelResults of the most recent run (for test harness)
DEBUG = os.environ.get("KDBG", "0") == "1"
DBG_T = int(os.environ.get("KDBG_T", "0"))   # which tile's mx to dump


# ---------------------------------------------------------------------------
# layout plan (depends only on im_len / s_len)
# ---------------------------------------------------------------------------

def make_plan(im_len, s_len):
    im_l = (np.asarray(im_len).astype(np.int64) - 1)    # 9..36 valid regions
    s_l = (np.asarray(s_len).astype(np.int64) - 3)      # 5..30 valid words
    # image slots: sort desc, deal rank-groups of 8 across cores
    order = np.argsort(-im_l, kind="stable")
    assign = order.reshape(32, NCORES)                  # [slot, core] -> b
    gmax = im_l[assign].max(axis=1)
    # quantize to mult of 4; strictly > im_l when im_l < LI (the zero-pad
    # row in-slot reproduces the reference max-with-0)
    slot_len = np.where(gmax == LI, LI, np.minimum(LI, 4 * ((gmax + 4) // 4)))
    slot_off = np.concatenate([[0], np.cumsum(slot_len)])
    SL = int(slot_off[-1])
    BIc = ((SL + 127) // 128) * 128
    NRT = BIc // 128
    # chunks: greedy pack slots into <=512-col PSUM banks, split at slots
    bounds = []
    cur_start = 0
    s0 = 0
    for r in range(32):
        if slot_off[r + 1] - cur_start > 512:
            bounds.append((cur_start, s0, r))
            cur_start = int(slot_off[r])
            s0 = r
    bounds.append((cur_start, s0, 32))
    chunks = []
    for noff, cs, se in bounds:
        runs = []
        r = cs
        while r < se:
            L = int(slot_len[r])
            cnt = 1
            while r + cnt < se and slot_len[r + cnt] == L:
                cnt += 1
            runs.append((int(slot_off[r]), L, cnt, r))
            r += cnt
        chunks.append((noff, int(slot_off[se] - noff), runs))
    # sentence packing: per half, all valid (c,j) rows then pad to 128
    cj_rows = []
    half_nt = []
    for h in range(2):
        for c in range(128 * h, 128 * h + 128):
            for j in range(int(s_l[c])):
                cj_rows.append((c, 1 + j))
        while len(cj_rows) % 128:
            cj_rows.append(None)
        half_nt.append(len(cj_rows) // 128)
    NT0 = half_nt[0]
    NT = half_nt[1]
    sig = (NT0, NT, SL, BIc, MM_F8, tuple(int(x) for x in slot_len))
    return dict(im_l=im_l, s_l=s_l, assign=assign, slot_len=slot_len,
                slot_off=slot_off, SL=SL, BIc=BIc, NRT=NRT, chunks=chunks,
                cj_rows=cj_rows, NT0=NT0, NT=NT, sig=sig)


# ---------------------------------------------------------------------------
# device program
# ---------------------------------------------------------------------------

def build_nc(plan):
    NT, NT0 = plan["NT"], plan["NT0"]
    NRT, BIc, SL = plan["NRT"], plan["BIc"], plan["SL"]
    chunks = plan["chunks"]

    nc = bacc.Bacc(None, target_bir_lowering=False, debug=False, num_devices=NCORES)

    MMDT = F8E4 if MM_F8 else BF16
    imr_e = nc.declare_dram_parameter("imr", [BIc, D], BF16, isOutput=False)
    srow_e = nc.declare_dram_parameter("srow", [NT, 128, D], BF16, isOutput=False)
    st_e = nc.declare_dram_parameter("st", [NT, 128, KC, 128], MMDT, isOutput=False)
    ident_e = nc.declare_dram_parameter("ident", [128, 128], F32, isOutput=False)
    gmat_e = nc.declare_dram_parameter("gmat", [128, NT * 128], BF16, isOutput=False)
    pos0_e = nc.declare_dram_parameter("pos0", [128, 32], F32, isOutput=False)
    pos1_e = nc.declare_dram_parameter("pos1", [128, 32], F32, isOutput=False)
    post0_e = nc.declare_dram_parameter("post0", [32, 128], F32, isOutput=False)
    post1_e = nc.declare_dram_parameter("post1", [32, 128], F32, isOutput=False)
    out_e = nc.declare_dram_parameter("out", [1, 1], F32, isOutput=True)
    if DEBUG:
        dbgsc_e = nc.declare_dram_parameter("dbgsc", [128, NT], F32, isOutput=True)
        dbgmx_e = nc.declare_dram_parameter("dbgmx", [128, 32], F32, isOutput=True)
        dbgS_e = nc.declare_dram_parameter("dbgS", [128, 64], F32, isOutput=True)
        dbgpay_e = nc.declare_dram_parameter("dbgpay", [128, 6], F32, isOutput=True)

    with tile.TileContext(nc) as tc:
        from contextlib import ExitStack

        with ExitStack() as ctx:
            dram = ctx.enter_context(tc.tile_pool(name="dram", bufs=1, space="DRAM"))
            const = ctx.enter_context(tc.tile_pool(name="const", bufs=1))
            small = ctx.enter_context(tc.tile_pool(name="small", bufs=1))
            stp = ctx.enter_context(tc.tile_pool(name="stp", bufs=10))
            mxp = ctx.enter_context(tc.tile_pool(name="mxp", bufs=1))
            prep = ctx.enter_context(tc.tile_pool(name="prep", bufs=3))
            # PSUM budget (8 banks): align ring 7 + S accumulator 1; the
            # epilogue scratch tiles borrow the align ring
            pal = ctx.enter_context(tc.tile_pool(name="pal", bufs=7, space="PSUM"))

            # DRAM scratch
            sync1_d = dram.tile([1, 1], F32, tag="sync1_d")
            sync8_d = dram.tile([NCORES, 1], F32, tag="sync8_d")
            pay_d = dram.tile([128, 6], F32, tag="pay_d")
            ag2_d = dram.tile([NCORES * 128, 6], F32, tag="ag2_d")

            def epi_psum(shape, name):
                return pal.tile(shape, F32, tag="al", name=name)

            ident = const.tile([128, 128], F32, tag="ident")
            nc.sync.dma_start(out=ident[:, :], in_=ident_e[:, :])

            # barrier collective: absorbs core launch skew early (hidden
            # under the head) so the payload AllGather's peer wait is short
            zz = small.tile([1, 1], F32, tag="zz")
            nc.gpsimd.memset(zz[:, :], 0.0)
            nc.sync.dma_start(out=sync1_d[:, :], in_=zz[:, :])
            nc.gpsimd.collective_compute(
                "AllGather", ALU.bypass,
                replica_groups=[list(range(NCORES))],
                ins=[sync1_d.opt()],
                outs=[sync8_d.opt()],
            )

            # ---- phase 1: im prep, one fused chain per row-tile ----
            ident_bf = const.tile([128, 128], BF16, tag="ident_bf")
            nc.scalar.copy(ident_bf[:, :], ident[:, :])
            epsb = const.tile([128, 1], F32, tag="epsb")
            nc.gpsimd.memset(epsb[:, :], EPS * EPS)
            # prewarm the Rsqrt ACT table before the squares occupy ACT
            tblw = small.tile([128, 1], F32, tag="tblw")
            _act_raw(nc.scalar, tblw[:, :], epsb[:, :], ACT.Rsqrt)

            imr_tiles = []
            for rt in range(NRT):
                imr_t = prep.tile([128, D], BF16, tag="imld", name="imr_t", bufs=NRT)
                nc.sync.dma_start(out=imr_t[:, :], in_=imr_e[128 * rt:128 * (rt + 1), :])
                imr_tiles.append(imr_t)

            # normalize+transpose fused on PE: the "ident" of the transpose
            # matmul is diag(IM_UPS/|row|), so raw bf16 tiles feed PE as soon
            # as their tiny rcp vector is ready; fp8 cast on the PSUM->imt
            # copy.  Squares alternate ACT/DVE so the two serial chains run
            # in parallel from the moment each imr DMA lands.
            imt = const.tile([128, KC * BIc], MMDT, tag="imt")
            nrm_scale = 1.0 / (IM_UPS * IM_UPS) if MM_F8 else 1.0
            for rt in range(NRT):
                imr_t = imr_tiles[rt]
                ssq = small.tile([128, 1], F32, tag=f"imssq{rt}", name="ssq")
                if rt % 2 == 0:
                    sq = prep.tile([128, D], F32, tag="imsq", name="sq")
                    nc.scalar.activation(sq[:, :], imr_t[:, :], ACT.Square,
                                         accum_out=ssq[:, :])
                else:
                    sq = prep.tile([128, D], F32, tag="imsqv", name="sq")
                    nc.vector.scalar_tensor_tensor(
                        out=sq[:, :], in0=imr_t[:, :], scalar=1.0,
                        in1=imr_t[:, :], op0=ALU.mult, op1=ALU.mult,
                        accum_out=ssq[:, :],
                    )
                rcp = small.tile([128, 1], F32, tag=f"imrcp{rt}")
                _act_raw(nc.scalar, rcp[:, :], ssq[:, :], ACT.Rsqrt,
                         bias=epsb[:, :], scale=nrm_scale)
                diag_rcp = small.tile([128, 128], BF16, tag=f"diagr{rt}")
                nc.scalar.mul(diag_rcp[:, :], ident_bf[:, :], mul=rcp[:, :])
                for k in range(KC):
                    pst = pal.tile([128, 128], F32, tag="al", name="pst")
                    nc.tensor.matmul(pst[:, :],
                                     lhsT=imr_t[:, 128 * k:128 * (k + 1)],
                                     rhs=diag_rcp[:, :], start=True, stop=True)
                    dst = imt[:, BIc * k + 128 * rt:BIc * k + 128 * (rt + 1)]
                    if k % 2 == 0:
                        nc.vector.tensor_copy(dst, pst[:, :])
                    else:
                        nc.scalar.copy(dst, pst[:, :])

            # ---- late consts (needed by G-matmuls / epilogue) ----
            gmat = const.tile([128, NT * 128], BF16, tag="gmat")
            nc.sync.dma_start(out=gmat[:, :], in_=gmat_e[:, :])
            pos0 = const.tile([128, 32], F32, tag="pos0")
            nc.sync.dma_start(out=pos0[:, :], in_=pos0_e[:, :])
            pos1 = const.tile([128, 32], F32, tag="pos1")
            nc.sync.dma_start(out=pos1[:, :], in_=pos1_e[:, :])
            post0 = const.tile([32, 128], F32, tag="post0")
            nc.sync.dma_start(out=post0[:, :], in_=post0_e[:, :])
            post1 = const.tile([32, 128], F32, tag="post1")
            nc.sync.dma_start(out=post1[:, :], in_=post1_e[:, :])
            margin128 = const.tile([128, 1], F32, tag="margin128")
            nc.gpsimd.memset(margin128[:, :], MARGIN)

            # ---- main loop over NT packed M-tiles ----
            # S halves share one PSUM bank: cols [0:32] half0, [32:64] half1
            psacc = ctx.enter_context(tc.tile_pool(name="psacc", bufs=1, space="PSUM"))
            s_acc = psacc.tile([128, 64], F32, tag="S", name="S")
            s_ps = [s_acc[:, 0:32], s_acc[:, 32:64]]

            posm = [pos0, pos1]
            payload = small.tile([128, 6], F32, tag="payload")
            snd = [small.tile([128, 32], F32, tag=f"snd{h}", name=f"snd{h}")
                   for h in range(2)]
            trash = small.tile([128, 32], F32, tag="trash")
            negm = [small.tile([128, 32], F32, tag=f"negm{h}", name=f"negm{h}")
                    for h in range(2)]
            nc.vector.tensor_scalar_mul(negm[0][:, :], pos0[:, :], NEG)
            nc.vector.tensor_scalar_mul(negm[1][:, :], pos1[:, :], NEG)
            posr = [small.tile([128, 32], F32R, tag=f"posr{h}", name=f"posr{h}")
                    for h in range(2)]
            nc.scalar.copy(posr[0][:, :], pos0[:, :])
            nc.scalar.copy(posr[1][:, :], pos1[:, :])
            postr = [small.tile([32, 128], F32R, tag=f"postr{h}", name=f"postr{h}")
                     for h in range(2)]
            nc.scalar.copy(postr[0][:, :], post0[:, :])
            nc.scalar.copy(postr[1][:, :], post1[:, :])
            rm = small.tile([32, 2], F32, tag="rm")

            def emit_stats_h(h):
                # diag extraction: accum_out = sum(S * pos) -> payload col 2+h
                nc.vector.scalar_tensor_tensor(
                    out=trash[:, :], in0=s_ps[h], scalar=1.0, in1=posm[h][:, :],
                    op0=ALU.mult, op1=ALU.mult, accum_out=payload[:, 2 + h:3 + h],
                )
                nc.vector.tensor_add(snd[h][:, :], s_ps[h], negm[h][:, :])
                nc.vector.tensor_reduce(out=payload[:, h:h + 1], in_=snd[h][:, :],
                                        axis=AX.X, op=ALU.max)
                stp_ps = epi_psum([32, 128], "stp_ps")
                nc.tensor.transpose(stp_ps[:, :], snd[h][:, :], ident[:, :])
                nc.vector.tensor_reduce(out=rm[:, h:h + 1], in_=stp_ps[:, :],
                                        axis=AX.X, op=ALU.max)

            ssqall = small.tile([128, NT], F32, tag="ssqall")
            sscale = small.tile([128, NT], F32, tag="sscale")

            # mx_r scaling (DVE) is decoupled from the G-matmul emission: the
            # scale for tile t is queued right after t's own max-reduces, so
            # it is ready one tile before PE reaches G(t)
            pending = {}
            pending_r = {}
            next_mul = [0]
            next_g = [0]

            def drain_muls(upto):
                while next_mul[0] <= upto:
                    u = next_mul[0]
                    mx_r = mxp.tile([128, 32], BF16, tag="mx_r", name="mx_r",
                                    bufs=12)
                    nc.vector.tensor_scalar(
                        out=mx_r[:, :], in0=pending[u][:, :],
                        scalar1=sscale[:, u:u + 1], scalar2=None, op0=ALU.mult,
                    )
                    pending_r[u] = mx_r
                    next_mul[0] += 1

            def drain_g(upto):
                while next_g[0] <= upto:
                    u = next_g[0]
                    h = 0 if u < NT0 else 1
                    nc.tensor.matmul(
                        s_ps[h],
                        lhsT=gmat[:, 128 * u:128 * (u + 1)],
                        rhs=pending_r.pop(u)[:, :],
                        start=(u == 0 or u == NT0),
                        stop=(u == NT0 - 1 or u == NT - 1),
                    )
                    next_g[0] += 1

            GSC = 6                 # sscale sqrt/rcp group size
            wm = [-1]               # highest tile with sscale emitted
            stats0 = [False]
            imt3 = imt.rearrange("p (k n) -> p k n", k=KC)
            for t in range(NT):
                srow_t = stp.tile([128, D], BF16, tag="srow", bufs=6)
                nc.sync.dma_start(out=srow_t[:, :], in_=srow_e[t, :, :])

                psc = [pal.tile([128, nw], F32, tag="al", name="ps")
                       for (_, nw, _) in chunks]
                mx = mxp.tile([128, 32], F32, tag="mx", name="mx", bufs=NT)
                st_t = stp.tile([128, KC * 128], MMDT, tag="st")
                nc.sync.dma_start(
                    out=st_t.rearrange("p (k c) -> p k c", k=KC),
                    in_=st_e[t, :, :, :],
                )
                if MM_F8:
                    # DoubleRow: 2 contraction k-tiles per instruction;
                    # chunk-inner keeps consecutive instrs on separate banks
                    st_t3 = st_t.rearrange("p (k c) -> p k c", k=KC)
                    for kp in range(KC // 2):
                        for ci, (noff, nw, _) in enumerate(chunks):
                            nc.tensor.matmul(
                                psc[ci][:, :],
                                lhsT=st_t3[:, 2 * kp:2 * kp + 2, :],
                                rhs=imt3[:, 2 * kp:2 * kp + 2, noff:noff + nw],
                                start=(kp == 0), stop=(kp == KC // 2 - 1),
                                perf_mode=mybir.MatmulPerfMode.DoubleRow,
                            )
                else:
                    # k-outer: consecutive matmuls alternate PSUM banks,
                    # hiding the accumulator read-modify-write turnaround
                    for k in range(KC):
                        for ci, (noff, nw, _) in enumerate(chunks):
                            nc.tensor.matmul(
                                psc[ci][:, :],
                                lhsT=st_t[:, 128 * k:128 * (k + 1)],
                                rhs=imt[:, BIc * k + noff:BIc * k + noff + nw],
                                start=(k == 0), stop=(k == KC - 1),
                            )
                for ci, (noff, nw, runs) in enumerate(chunks):
                    for (off, L, cnt, slot0) in runs:
                        nc.vector.tensor_reduce(
                            out=mx[:, slot0:slot0 + cnt],
                            in_=psc[ci][:, off - noff:off - noff + cnt * L]
                            .rearrange("p (g i) -> p g i", i=L),
                            axis=AX.X, op=ALU.max,
                        )
                # per-tile s row norms (redundant per core; no collective) on
                # ACT, which is otherwise idle mid-loop; emitted after the
                # reduces so early-loop queues aren't blocked on srow DMAs
                strash = stp.tile([128, D], F32, tag="strash2", bufs=2)
                nc.scalar.activation(strash[:, :], srow_t[:, :], ACT.Square,
                                     accum_out=ssqall[:, t:t + 1])
                pending[t] = mx
                if DEBUG and t == DBG_T:
                    nc.sync.dma_start(out=dbgmx_e[:, :], in_=mx[:, :])

                if (t + 1) % GSC == 0 or t == NT - 1:
                    lo = wm[0] + 1
                    # sscale = 1/(IM_UPS*S_UPS*|s|) in one ACT op (undoes the
                    # fp8 upscales); bias keeps zero pad rows finite
                    _act_raw(nc.scalar, sscale[:, lo:t + 1], ssqall[:, lo:t + 1],
                             ACT.Rsqrt, bias=epsb[:, :],
                             scale=(IM_UPS * S_UPS) ** 2 if MM_F8 else 1.0)
                    wm[0] = t
                drain_muls(min(t, wm[0]))
                drain_g(min(t - DRAIN_LAG, next_mul[0] - 1))
                if not stats0[0] and next_g[0] > NT0 - 1 and t >= NT0 + 1:
                    emit_stats_h(0)
                    stats0[0] = True
            drain_muls(NT - 1)
            drain_g(NT - 1)
            if DEBUG:
                nc.sync.dma_start(out=dbgsc_e[:, :], in_=sscale[:, :])

            # ---- epilogue: half-1 stats + AllGather + final ----
            emit_stats_h(1)
            if DEBUG:
                sdump = small.tile([128, 64], F32, tag="sdump")
                nc.vector.tensor_copy(sdump[:, 0:32], s_ps[0])
                nc.vector.tensor_copy(sdump[:, 32:64], s_ps[1])
                nc.sync.dma_start(out=dbgS_e[:, :], in_=sdump[:, :])
            # diag per slot: pair pay col 2 with pos0 and col 3 with pos1 in
            # SEPARATE rhs columns (a core owns images of both halves, so a
            # shared 2-col rhs would contaminate: diag0[g] + diag1[g])
            dcol4 = small.tile([128, 4], F32R, tag="dcol4")
            nc.scalar.copy(dcol4[:, 0:1], payload[:, 2:3])
            nc.scalar.mul(dcol4[:, 1:2], payload[:, 2:3], mul=0.0)
            nc.scalar.copy(dcol4[:, 2:3], payload[:, 3:4])
            nc.scalar.mul(dcol4[:, 3:4], payload[:, 3:4], mul=0.0)
            rowmax = small.tile([32, 1], F32, tag="rowmax")
            nc.vector.tensor_max(rowmax[:, :], rm[:, 0:1], rm[:, 1:2])
            dfree_ps = epi_psum([32, 2], "dfree_ps")
            nc.tensor.matmul(dfree_ps[:, :], lhsT=posr[0][:, :],
                             rhs=dcol4[:, 0:2], start=True, stop=False)
            nc.tensor.matmul(dfree_ps[:, :], lhsT=posr[1][:, :],
                             rhs=dcol4[:, 2:4], start=False, stop=True)
            dfree_sb = small.tile([32, 1], F32, tag="dfree_sb")
            nc.scalar.copy(dfree_sb[:, :], dfree_ps[:, 0:1])
            rh_pre = small.tile([32, 2], F32, tag="rh_pre")
            nc.gpsimd.memset(rh_pre[:, :], 0.0)
            nc.vector.tensor_sub(rh_pre[:, 0:1], rowmax[:, :], dfree_sb[:, :])
            rowhinge = small.tile([32, 2], F32R, tag="rowhinge")
            nc.scalar.activation(rowhinge[:, :], rh_pre[:, :], ACT.Relu,
                                 bias=margin128[0:32, :])
            for h in range(2):
                rh_ps = epi_psum([128, 2], "rh_ps")
                nc.tensor.matmul(rh_ps[:, :], lhsT=postr[h][:, :],
                                 rhs=rowhinge[:, :], start=True, stop=True)
                nc.scalar.copy(payload[:, 4 + h:5 + h], rh_ps[:, 0:1])

            # payload -> DRAM (one DMA, contiguous per partition) -> AllGather
            if DEBUG:
                nc.sync.dma_start(out=dbgpay_e[:, :], in_=payload[:, :])
            nc.sync.dma_start(out=pay_d[:, :], in_=payload[:, :])
            nc.gpsimd.collective_compute(
                "AllGather", ALU.bypass,
                replica_groups=[list(range(NCORES))],
                ins=[pay_d.opt()],
                outs=[ag2_d.opt()],
            )

            # final combine (identical on every core): strided reload puts
            # sentences on partitions, (stat, core) along free; DVE reduces
            agg_in = small.tile([128, 6 * NCORES], F32, tag="agg_in")
            nc.sync.dma_start(
                out=agg_in.rearrange("p (c m) -> p c m", m=NCORES),
                in_=ag2_d.rearrange("(m p) c -> p c m", m=NCORES),
            )
            agv = agg_in.rearrange("p (c m) -> p c m", m=NCORES)
            agg = small.tile([128, 6], F32, tag="agg")
            nc.vector.tensor_reduce(out=agg[:, 0:2], in_=agv[:, 0:2, :],
                                    axis=AX.X, op=ALU.max)
            nc.vector.tensor_reduce(out=agg[:, 2:6], in_=agv[:, 2:6, :],
                                    axis=AX.X, op=ALU.add)
            ch2 = small.tile([128, 2], F32, tag="ch2")
            nc.vector.tensor_sub(ch2[:, :], agg[:, 0:2], agg[:, 2:4])
            hing4 = small.tile([128, 4], F32, tag="hing4")
            nc.scalar.activation(hing4[:, 0:2], ch2[:, :], ACT.Relu,
                                 bias=margin128[:, :])
            nc.scalar.copy(hing4[:, 2:4], agg[:, 4:6])
            psum128 = small.tile([128, 1], F32, tag="psum128")
            nc.vector.tensor_reduce(out=psum128[:, :], in_=hing4[:, :],
                                    axis=AX.X, op=ALU.add)
            lsum_ps = epi_psum([1, 128], "lsum_ps")
            nc.tensor.transpose(lsum_ps[:, :], psum128[:, :], ident[:, :])
            loss = small.tile([1, 1], F32, tag="loss")
            nc.vector.tensor_reduce(out=loss[:, :], in_=lsum_ps[:, :], axis=AX.X,
                                    op=ALU.add)
            nc.sync.dma_start(out=out_e[:, :], in_=loss[:, :])

    nc.finalize()
    return nc


# ---------------------------------------------------------------------------
# host side
# ---------------------------------------------------------------------------

def build_in_maps(plan, im_set, s_seq):
    im_set = np.asarray(im_set, dtype=np.float32)
    s_seq = np.asarray(s_seq, dtype=np.float32)
    NT, NT0, BIc = plan["NT"], plan["NT0"], plan["BIc"]
    cj = plan["cj_rows"]
    CJc = NT * 128

    s_rows = np.zeros((CJc, D), np.float32)
    idx = [i for i, row in enumerate(cj) if row is not None]
    cs = np.array([cj[i][0] for i in idx])
    js = np.array([cj[i][1] for i in idx])
    s_rows[idx] = s_seq[cs, js]
    st_f32 = np.ascontiguousarray(
        s_rows.reshape(NT, 128, KC, 128).transpose(0, 3, 2, 1))
    if MM_F8:
        st = np.clip(st_f32 * S_UPS, -240.0, 240.0).astype(F8)
    else:
        st = st_f32.astype(BF)
    srow = s_rows.reshape(NT, 128, D).astype(BF)

    gmat = np.zeros((128, NT * 128), BF)
    for t in range(NT):
        h = 0 if t < NT0 else 1
        for p in range(128):
            row = cj[128 * t + p]
            if row is not None:
                gmat[p, 128 * t + (row[0] - 128 * h)] = 1.0
    ident = np.eye(128, dtype=np.float32)

    in_maps = []
    for m in range(NCORES):
        imr = np.zeros((BIc, D), BF)
        pos0 = np.zeros((128, 32), np.float32)
        pos1 = np.zeros((128, 32), np.float32)
        for r in range(32):
            g = int(plan["assign"][r, m])
            l = int(plan["im_l"][g])
            o = int(plan["slot_off"][r])
            imr[o:o + l] = im_set[g, 1:1 + l]
            (pos0 if g < 128 else pos1)[g % 128, r] = 1.0
        in_maps.append({
            "imr": imr,
            "srow": srow,
            "st": st,
            "ident": ident,
            "gmat": gmat,
            "pos0": pos0,
            "pos1": pos1,
            "post0": np.ascontiguousarray(pos0.T),
            "post1": np.ascontiguousarray(pos1.T),
        })
    return in_maps


_NC_CACHE = {}


def kernel(im_set, s_seq, im_len, s_len):
    global LAST_RESULT
    plan = make_plan(im_len, s_len)
    key = plan["sig"]
    nc = _NC_CACHE.get(key)
    in_maps = build_in_maps(plan, im_set, s_seq)
    if nc is None:
        nc = build_nc(plan)
        _NC_CACHE[key] = nc
        # first executions of a fresh NEFF carry upload/launch-skew cost
        # (cores desynchronize by >100us); warm it up
        for _ in range(3):
            run_bass_kernel_spmd(nc, in_maps, core_ids=list(range(NCORES)))
    res = run_bass_kernel_spmd(nc, in_maps, core_ids=list(range(NCORES)))
    LAST_RESULT = res
    out = np.asarray(res.results[0]["out"], dtype=np.float32).reshape(())
    return out


# revision 95
# speedup vs baseline: 2.3464x; 1.1201x over previous
"""Distributed Trainium2 Bass kernel for AlignmentContrastiveLoss (packed).

Reference computation (B=256, L_im=37, L_s=33, D=1024):
    im  = l2norm(im_set)[:, 1:, :]   masked by im_len-1     [B, 36, D]
    s   = l2norm(s_seq)[:, 1:-2, :]  masked by s_len-3      [B, 30, D]
    align[b,c,i,j] = im[b,i] . s[c,j]   (masked entries -> 0)
    scores[b,c] = sum_j max_i align[b,c,i,j]
    loss = sum_b relu(M + max_{c!=b} scores[b,c] - scores[b,b])
         + sum_c relu(M + max_{b!=c} scores[b,c] - scores[c,c])

Sparsity exploitation (the big win vs a dense kernel): only valid im
regions / s words are ever loaded or multiplied.
  * s side: all valid (c,j) rows are packed densely (per 128-sentence
    half, zero-padded to 128-row tiles) -> NT ~ 36 instead of 60 tiles.
    Invalid words contribute exactly 0 to scores, so dropping them is
    exact; the host-built 0/1 G matrix maps packed rows -> sentences.
  * im side: images are sorted by region count and dealt round-robin
    (rank r -> core r%8, slot r//8) so all 8 cores share one compiled
    slot profile; slot lengths are the per-group max quantized to
    multiples of 4 (<= 7 distinct lengths -> few DVE reduce runs).
    Images with im_l < 36 get >= 1 zero pad row in their slot, which
    reproduces the reference's max-with-0 semantics exactly.
  * per-core matmul: s packed rows stationary (bf16, host-cast),
    normalized im rows moving; max-over-i from PSUM on DVE; 1/|s| folded
    in post-max on ACT; 0/1 G matmuls accumulate scoresT [128 x 32] per
    half; s norms are computed on-device from a sharded row slice and
    AllGathered; final per-core stats AllGathered (768 floats) and the
    scalar loss computed redundantly on every core.
"""

import math
import os
import sys

import numpy as np

for _p in ("/opt/trn_rl_repo", "/root/.axon_site/_ro/trn_rl_repo"):
    if os.path.isdir(_p) and _p not in sys.path:
        sys.path.append(_p)

import ml_dtypes

import concourse.bass as bass
import concourse.mybir as mybir
import concourse.tile as tile
from concourse import bacc
from concourse.bass_utils import run_bass_kernel_spmd


def _act_raw(eng, out, in_, func, bias=0.0, scale=1.0):
    """Emit an InstActivation directly (nc.scalar.activation refuses
    Reciprocal/Rsqrt on accuracy grounds; our tolerance is ~2e-2, so the
    table approximation is more than fine here)."""
    ins = [eng.lower_ap(in_)]
    b = eng.bass.const_aps.scalar_like(bias, in_) if isinstance(bias, float) else bias
    for arg in (b, scale, 0.0):
        if isinstance(arg, (int, float)):
            ins.append(mybir.ImmediateValue(dtype=mybir.dt.float32, value=float(arg)))
        else:
            ins.append(eng.lower_ap(arg))
    return eng.add_instruction(
        mybir.InstActivation(
            name=eng.bass.get_next_instruction_name(),
            func=func, ins=ins, outs=[eng.lower_ap(out)],
        )
    )


def _ensure_axon_hooks():
    """Some agent images ship an ``antenv`` without ``axon_hooks``, but
    bass_utils hard-imports it when trace=True.  Provide the registry and,
    when libaxon_pjrt.so is available, the real NTFF profile hook."""
    import types

    try:
        import antenv.axon_hooks  # noqa: F401
        return
    except ImportError:
        pass
    try:
        import antenv
    except ImportError:
        return
    mod = types.ModuleType("antenv.axon_hooks")
    mod._hook = None
    mod.set_axon_ntff_profile_hook = lambda h: setattr(mod, "_hook", h)
    mod.get_axon_ntff_profile_hook = lambda: mod._hook
    sys.modules["antenv.axon_hooks"] = mod
    antenv.axon_hooks = mod
    so_path = "/opt/axon/libaxon_pjrt.so"
    try:
        import trn_agent_boot.trn_boot as _tb
        if os.path.exists(so_path):
            mod._hook = _tb._ntff_profile_via_ctypes(so_path)
    except Exception:
        pass


_ensure_axon_hooks()

F32 = mybir.dt.float32
F32R = mybir.dt.float32r
BF16 = mybir.dt.bfloat16
F8E4 = mybir.dt.float8e4
I32 = mybir.dt.int32
AX = mybir.AxisListType
ALU = mybir.AluOpType
ACT = mybir.ActivationFunctionType
BF = ml_dtypes.bfloat16
F8 = ml_dtypes.float8_e4m3

# fp8 alignment matmuls (DoubleRow: 2 contraction tiles / instruction).
# im rows scaled x128, s rows x32 pre-quantization; 1/4096 folded into the
# post-max 1/|s| scale, so downstream math is unchanged.
MM_F8 = os.environ.get("KF8", "1") == "1"
IM_UPS = 128.0
S_UPS = 32.0

NCORES = 8
B, LI, LS, D = 256, 36, 30, 1024
KC = D // 128               # 8 contraction chunks
MARGIN, EPS, NEG = 0.2, 1e-12, -1.0e9

DRAIN_LAG = 2               # G-matmul for tile t emitted at loop step t+2

LAST_RESULT = None  # BassKernelResults of the most recent run (for test harness)
DEBUG = os.environ.get("KDBG", "0") == "1"
DBG_T = int(os.environ.get("KDBG_T", "0"))   # which tile's mx to dump


# ---------------------------------------------------------------------------
# layout plan (depends only on im_len / s_len)
# ---------------------------------------------------------------------------

def make_plan(im_len, s_len):
    im_l = (np.asarray(im_len).astype(np.int64) - 1)    # 9..36 valid regions
    s_l = (np.asarray(s_len).astype(np.int64) - 3)      # 5..30 valid words
    # image slots: sort desc, deal rank-groups of 8 across cores
    order = np.argsort(-im_l, kind="stable")
    assign = order.reshape(32, NCORES)                  # [slot, core] -> b
    gmax = im_l[assign].max(axis=1)
    # quantize to mult of 4; strictly > im_l when im_l < LI (the zero-pad
    # row in-slot reproduces the reference max-with-0)
    slot_len = np.where(gmax == LI, LI, np.minimum(LI, 4 * ((gmax + 4) // 4)))
    slot_off = np.concatenate([[0], np.cumsum(slot_len)])
    SL = int(slot_off[-1])
    BIc = ((SL + 127) // 128) * 128
    NRT = BIc // 128
    # chunks: pack slots into <=512-col PSUM banks.  Prefer cutting at a
    # boundary between equal-length runs (keeps one max-reduce per run and
    # balances the chunk widths); fall back to greedy slot cuts.
    run_bnd = [0]
    for r in range(1, 32):
        if slot_len[r] != slot_len[r - 1]:
            run_bnd.append(r)
    cut = None
    if SL <= 1024:
        for r in reversed(run_bnd[1:]):
            if slot_off[r] <= 512 and SL - slot_off[r] <= 512:
                cut = r
                break
    if cut is not None:
        bounds = [(0, 0, cut), (int(slot_off[cut]), cut, 32)]
    else:
        bounds = []
        cur_start = 0
        s0 = 0
        for r in range(32):
            if slot_off[r + 1] - cur_start > 512:
                bounds.append((cur_start, s0, r))
                cur_start = int(slot_off[r])
                s0 = r
        bounds.append((cur_start, s0, 32))
    chunks = []
    for noff, cs, se in bounds:
        runs = []
        r = cs
        while r < se:
            L = int(slot_len[r])
            cnt = 1
            while r + cnt < se and slot_len[r + cnt] == L:
                cnt += 1
            runs.append((int(slot_off[r]), L, cnt, r))
            r += cnt
        chunks.append((noff, int(slot_off[se] - noff), runs))
    # sentence packing: per half, all valid (c,j) rows then pad to 128
    cj_rows = []
    half_nt = []
    for h in range(2):
        for c in range(128 * h, 128 * h + 128):
            for j in range(int(s_l[c])):
                cj_rows.append((c, 1 + j))
        while len(cj_rows) % 128:
            cj_rows.append(None)
        half_nt.append(len(cj_rows) // 128)
    NT0 = half_nt[0]
    NT = half_nt[1]
    sig = (NT0, NT, SL, BIc, MM_F8, tuple(int(x) for x in slot_len))
    return dict(im_l=im_l, s_l=s_l, assign=assign, slot_len=slot_len,
                slot_off=slot_off, SL=SL, BIc=BIc, NRT=NRT, chunks=chunks,
                cj_rows=cj_rows, NT0=NT0, NT=NT, sig=sig)


# ---------------------------------------------------------------------------
# device program
# ---------------------------------------------------------------------------

def build_nc(plan):
    NT, NT0 = plan["NT"], plan["NT0"]
    NRT, BIc, SL = plan["NRT"], plan["BIc"], plan["SL"]
    chunks = plan["chunks"]

    nc = bacc.Bacc(None, target_bir_lowering=False, debug=False, num_devices=NCORES)

    MMDT = F8E4 if MM_F8 else BF16
    imr_e = nc.declare_dram_parameter("imr", [BIc, D], BF16, isOutput=False)
    srow_e = nc.declare_dram_parameter("srow", [NT, 128, D], BF16, isOutput=False)
    st_e = nc.declare_dram_parameter("st", [NT, 128, KC, 128], MMDT, isOutput=False)
    ident_e = nc.declare_dram_parameter("ident", [128, 128], F32, isOutput=False)
    gmat_e = nc.declare_dram_parameter("gmat", [128, NT * 128], BF16, isOutput=False)
    pos0_e = nc.declare_dram_parameter("pos0", [128, 32], F32, isOutput=False)
    pos1_e = nc.declare_dram_parameter("pos1", [128, 32], F32, isOutput=False)
    post0_e = nc.declare_dram_parameter("post0", [32, 128], F32, isOutput=False)
    post1_e = nc.declare_dram_parameter("post1", [32, 128], F32, isOutput=False)
    out_e = nc.declare_dram_parameter("out", [1, 1], F32, isOutput=True)
    if DEBUG:
        dbgsc_e = nc.declare_dram_parameter("dbgsc", [128, NT], F32, isOutput=True)
        dbgmx_e = nc.declare_dram_parameter("dbgmx", [128, 32], F32, isOutput=True)
        dbgS_e = nc.declare_dram_parameter("dbgS", [128, 64], F32, isOutput=True)
        dbgpay_e = nc.declare_dram_parameter("dbgpay", [128, 6], F32, isOutput=True)

    with tile.TileContext(nc) as tc:
        from contextlib import ExitStack

        with ExitStack() as ctx:
            dram = ctx.enter_context(tc.tile_pool(name="dram", bufs=1, space="DRAM"))
            const = ctx.enter_context(tc.tile_pool(name="const", bufs=1))
            small = ctx.enter_context(tc.tile_pool(name="small", bufs=1))
            stp = ctx.enter_context(tc.tile_pool(name="stp", bufs=10))
            mxp = ctx.enter_context(tc.tile_pool(name="mxp", bufs=1))
            prep = ctx.enter_context(tc.tile_pool(name="prep", bufs=3))
            # PSUM budget (8 banks): align ring 7 + S accumulator 1; the
            # epilogue scratch tiles borrow the align ring
            pal = ctx.enter_context(tc.tile_pool(name="pal", bufs=7, space="PSUM"))

            # DRAM scratch
            sync1_d = dram.tile([1, 1], F32, tag="sync1_d")
            sync8_d = dram.tile([NCORES, 1], F32, tag="sync8_d")
            pay_d = dram.tile([128, 6], F32, tag="pay_d")
            ag2_d = dram.tile([NCORES * 128, 6], F32, tag="ag2_d")

            def epi_psum(shape, name):
                return pal.tile(shape, F32, tag="al", name=name)

            ident = const.tile([128, 128], F32, tag="ident")
            nc.sync.dma_start(out=ident[:, :], in_=ident_e[:, :])

            # barrier collective: absorbs core launch skew early (hidden
            # under the head) so the payload AllGather's peer wait is short
            zz = small.tile([1, 1], F32, tag="zz")
            nc.gpsimd.memset(zz[:, :], 0.0)
            nc.sync.dma_start(out=sync1_d[:, :], in_=zz[:, :])
            nc.gpsimd.collective_compute(
                "AllGather", ALU.bypass,
                replica_groups=[list(range(NCORES))],
                ins=[sync1_d.opt()],
                outs=[sync8_d.opt()],
            )

            # ---- phase 1: im prep, one fused chain per row-tile ----
            ident_bf = const.tile([128, 128], BF16, tag="ident_bf")
            nc.scalar.copy(ident_bf[:, :], ident[:, :])
            epsb = const.tile([128, 1], F32, tag="epsb")
            nc.gpsimd.memset(epsb[:, :], EPS * EPS)
            # prewarm the Rsqrt ACT table before the squares occupy ACT
            tblw = small.tile([128, 1], F32, tag="tblw")
            _act_raw(nc.scalar, tblw[:, :], epsb[:, :], ACT.Rsqrt)

            imr_tiles = []
            for rt in range(NRT):
                imr_t = prep.tile([128, D], BF16, tag="imld", name="imr_t", bufs=NRT)
                nc.sync.dma_start(out=imr_t[:, :], in_=imr_e[128 * rt:128 * (rt + 1), :])
                imr_tiles.append(imr_t)

            # normalize+transpose fused on PE: the "ident" of the transpose
            # matmul is diag(IM_UPS/|row|), so raw bf16 tiles feed PE as soon
            # as their tiny rcp vector is ready; fp8 cast on the PSUM->imt
            # copy.  Squares alternate ACT/DVE so the two serial chains run
            # in parallel from the moment each imr DMA lands.
            imt = const.tile([128, KC * BIc], MMDT, tag="imt")
            nrm_scale = 1.0 / (IM_UPS * IM_UPS) if MM_F8 else 1.0
            for rt in range(NRT):
                imr_t = imr_tiles[rt]
                ssq = small.tile([128, 1], F32, tag=f"imssq{rt}", name="ssq")
                if rt % 2 == 0:
                    sq = prep.tile([128, D], F32, tag="imsq", name="sq")
                    nc.scalar.activation(sq[:, :], imr_t[:, :], ACT.Square,
                                         accum_out=ssq[:, :])
                else:
                    sq = prep.tile([128, D], F32, tag="imsqv", name="sq")
                    nc.vector.scalar_tensor_tensor(
                        out=sq[:, :], in0=imr_t[:, :], scalar=1.0,
                        in1=imr_t[:, :], op0=ALU.mult, op1=ALU.mult,
                        accum_out=ssq[:, :],
                    )
                rcp = small.tile([128, 1], F32, tag=f"imrcp{rt}")
                _act_raw(nc.scalar, rcp[:, :], ssq[:, :], ACT.Rsqrt,
                         bias=epsb[:, :], scale=nrm_scale)
                diag_rcp = small.tile([128, 128], BF16, tag=f"diagr{rt}")
                nc.scalar.mul(diag_rcp[:, :], ident_bf[:, :], mul=rcp[:, :])
                for k in range(KC):
                    pst = pal.tile([128, 128], F32, tag="al", name="pst")
                    nc.tensor.matmul(pst[:, :],
                                     lhsT=imr_t[:, 128 * k:128 * (k + 1)],
                                     rhs=diag_rcp[:, :], start=True, stop=True)
                    dst = imt[:, BIc * k + 128 * rt:BIc * k + 128 * (rt + 1)]
                    if k % 2 == 0:
                        nc.vector.tensor_copy(dst, pst[:, :])
                    else:
                        nc.scalar.copy(dst, pst[:, :])

            # ---- late consts (needed by G-matmuls / epilogue) ----
            gmat = const.tile([128, NT * 128], BF16, tag="gmat")
            nc.sync.dma_start(out=gmat[:, :], in_=gmat_e[:, :])
            pos0 = const.tile([128, 32], F32, tag="pos0")
            nc.sync.dma_start(out=pos0[:, :], in_=pos0_e[:, :])
            pos1 = const.tile([128, 32], F32, tag="pos1")
            nc.sync.dma_start(out=pos1[:, :], in_=pos1_e[:, :])
            post0 = const.tile([32, 128], F32, tag="post0")
            nc.sync.dma_start(out=post0[:, :], in_=post0_e[:, :])
            post1 = const.tile([32, 128], F32, tag="post1")
            nc.sync.dma_start(out=post1[:, :], in_=post1_e[:, :])
            margin128 = const.tile([128, 1], F32, tag="margin128")
            nc.gpsimd.memset(margin128[:, :], MARGIN)

            # ---- main loop over NT packed M-tiles ----
            # S halves share one PSUM bank: cols [0:32] half0, [32:64] half1
            psacc = ctx.enter_context(tc.tile_pool(name="psacc", bufs=1, space="PSUM"))
            s_acc = psacc.tile([128, 64], F32, tag="S", name="S")
            s_ps = [s_acc[:, 0:32], s_acc[:, 32:64]]

            posm = [pos0, pos1]
            payload = small.tile([128, 6], F32, tag="payload")
            snd = [small.tile([128, 32], F32, tag=f"snd{h}", name=f"snd{h}")
                   for h in range(2)]
            trash = small.tile([128, 32], F32, tag="trash")
            negm = [small.tile([128, 32], F32, tag=f"negm{h}", name=f"negm{h}")
                    for h in range(2)]
            nc.vector.tensor_scalar_mul(negm[0][:, :], pos0[:, :], NEG)
            nc.vector.tensor_scalar_mul(negm[1][:, :], pos1[:, :], NEG)
            posr = [small.tile([128, 32], F32R, tag=f"posr{h}", name=f"posr{h}")
                    for h in range(2)]
            nc.scalar.copy(posr[0][:, :], pos0[:, :])
            nc.scalar.copy(posr[1][:, :], pos1[:, :])
            postr = [small.tile([32, 128], F32R, tag=f"postr{h}", name=f"postr{h}")
                     for h in range(2)]
            nc.scalar.copy(postr[0][:, :], post0[:, :])
            nc.scalar.copy(postr[1][:, :], post1[:, :])
            rm = small.tile([32, 2], F32, tag="rm")

            def emit_stats_h(h):
                # diag extraction: accum_out = sum(S * pos) -> payload col 2+h
                nc.vector.scalar_tensor_tensor(
                    out=trash[:, :], in0=s_ps[h], scalar=1.0, in1=posm[h][:, :],
                    op0=ALU.mult, op1=ALU.mult, accum_out=payload[:, 2 + h:3 + h],
                )
                nc.vector.tensor_add(snd[h][:, :], s_ps[h], negm[h][:, :])
                nc.vector.tensor_reduce(out=payload[:, h:h + 1], in_=snd[h][:, :],
                                        axis=AX.X, op=ALU.max)
                stp_ps = epi_psum([32, 128], "stp_ps")
                nc.tensor.transpose(stp_ps[:, :], snd[h][:, :], ident[:, :])
                nc.vector.tensor_reduce(out=rm[:, h:h + 1], in_=stp_ps[:, :],
                                        axis=AX.X, op=ALU.max)

            ssqall = small.tile([128, NT], F32, tag="ssqall")
            sscale = small.tile([128, NT], F32, tag="sscale")

            # mx_r scaling (DVE) is decoupled from the G-matmul emission: the
            # scale for tile t is queued right after t's own max-reduces, so
            # it is ready one tile before PE reaches G(t)
            pending = {}
            pending_r = {}
            next_mul = [0]
            next_g = [0]

            def drain_muls(upto):
                while next_mul[0] <= upto:
                    u = next_mul[0]
                    mx_r = mxp.tile([128, 32], BF16, tag="mx_r", name="mx_r",
                                    bufs=12)
                    nc.vector.tensor_scalar(
                        out=mx_r[:, :], in0=pending[u][:, :],
                        scalar1=sscale[:, u:u + 1], scalar2=None, op0=ALU.mult,
                    )
                    pending_r[u] = mx_r
                    next_mul[0] += 1

            def drain_g(upto):
                while next_g[0] <= upto:
                    u = next_g[0]
                    h = 0 if u < NT0 else 1
                    nc.tensor.matmul(
                        s_ps[h],
                        lhsT=gmat[:, 128 * u:128 * (u + 1)],
                        rhs=pending_r.pop(u)[:, :],
                        start=(u == 0 or u == NT0),
                        stop=(u == NT0 - 1 or u == NT - 1),
                    )
                    next_g[0] += 1

            GSC = 6                 # sscale sqrt/rcp group size
            wm = [-1]               # highest tile with sscale emitted
            stats0 = [False]
            imt3 = imt.rearrange("p (k n) -> p k n", k=KC)
            for t in range(NT):
                srow_t = stp.tile([128, D], BF16, tag="srow", bufs=6)
                nc.sync.dma_start(out=srow_t[:, :], in_=srow_e[t, :, :])

                psc = [pal.tile([128, nw], F32, tag="al", name="ps")
                       for (_, nw, _) in chunks]
                mx = mxp.tile([128, 32], F32, tag="mx", name="mx", bufs=NT)
                st_t = stp.tile([128, KC * 128], MMDT, tag="st")
                nc.sync.dma_start(
                    out=st_t.rearrange("p (k c) -> p k c", k=KC),
                    in_=st_e[t, :, :, :],
                )
                if MM_F8:
                    # DoubleRow: 2 contraction k-tiles per instruction;
                    # chunk-inner keeps consecutive instrs on separate banks
                    st_t3 = st_t.rearrange("p (k c) -> p k c", k=KC)
                    for kp in range(KC // 2):
                        for ci, (noff, nw, _) in enumerate(chunks):
                            nc.tensor.matmul(
                                psc[ci][:, :],
                                lhsT=st_t3[:, 2 * kp:2 * kp + 2, :],
                                rhs=imt3[:, 2 * kp:2 * kp + 2, noff:noff + nw],
                                start=(kp == 0), stop=(kp == KC // 2 - 1),
                                perf_mode=mybir.MatmulPerfMode.DoubleRow,
                            )
                else:
                    # k-outer: consecutive matmuls alternate PSUM banks,
                    # hiding the accumulator read-modify-write turnaround
                    for k in range(KC):
                        for ci, (noff, nw, _) in enumerate(chunks):
                            nc.tensor.matmul(
                                psc[ci][:, :],
                                lhsT=st_t[:, 128 * k:128 * (k + 1)],
                                rhs=imt[:, BIc * k + noff:BIc * k + noff + nw],
                                start=(k == 0), stop=(k == KC - 1),
                            )
                for ci, (noff, nw, runs) in enumerate(chunks):
                    for (off, L, cnt, slot0) in runs:
                        nc.vector.tensor_reduce(
                            out=mx[:, slot0:slot0 + cnt],
                            in_=psc[ci][:, off - noff:off - noff + cnt * L]
                            .rearrange("p (g i) -> p g i", i=L),
                            axis=AX.X, op=ALU.max,
                        )
                # per-tile s row norms (redundant per core; no collective) on
                # ACT, which is otherwise idle mid-loop; emitted after the
                # reduces so early-loop queues aren't blocked on srow DMAs
                strash = stp.tile([128, D], F32, tag="strash2", bufs=2)
                nc.scalar.activation(strash[:, :], srow_t[:, :], ACT.Square,
                                     accum_out=ssqall[:, t:t + 1])
                pending[t] = mx
                if DEBUG and t == DBG_T:
                    nc.sync.dma_start(out=dbgmx_e[:, :], in_=mx[:, :])

                if (t + 1) % GSC == 0 or t == NT - 1:
                    lo = wm[0] + 1
                    # sscale = 1/(IM_UPS*S_UPS*|s|) in one ACT op (undoes the
                    # fp8 upscales); bias keeps zero pad rows finite
                    _act_raw(nc.scalar, sscale[:, lo:t + 1], ssqall[:, lo:t + 1],
                             ACT.Rsqrt, bias=epsb[:, :],
                             scale=(IM_UPS * S_UPS) ** 2 if MM_F8 else 1.0)
                    wm[0] = t
                drain_muls(min(t, wm[0]))
                drain_g(min(t - DRAIN_LAG, next_mul[0] - 1))
                if not stats0[0] and next_g[0] > NT0 - 1 and t >= NT0 + 1:
                    emit_stats_h(0)
                    stats0[0] = True
            drain_muls(NT - 1)
            drain_g(NT - 1)
            if DEBUG:
                nc.sync.dma_start(out=dbgsc_e[:, :], in_=sscale[:, :])

            # ---- epilogue: half-1 stats + AllGather + final ----
            emit_stats_h(1)
            if DEBUG:
                sdump = small.tile([128, 64], F32, tag="sdump")
                nc.vector.tensor_copy(sdump[:, 0:32], s_ps[0])
                nc.vector.tensor_copy(sdump[:, 32:64], s_ps[1])
                nc.sync.dma_start(out=dbgS_e[:, :], in_=sdump[:, :])
            # diag per slot: pair pay col 2 with pos0 and col 3 with pos1 in
            # SEPARATE rhs columns (a core owns images of both halves, so a
            # shared 2-col rhs would contaminate: diag0[g] + diag1[g])
            dcol4 = small.tile([128, 4], F32R, tag="dcol4")
            nc.scalar.copy(dcol4[:, 0:1], payload[:, 2:3])
            nc.scalar.mul(dcol4[:, 1:2], payload[:, 2:3], mul=0.0)
            nc.scalar.copy(dcol4[:, 2:3], payload[:, 3:4])
            nc.scalar.mul(dcol4[:, 3:4], payload[:, 3:4], mul=0.0)
            rowmax = small.tile([32, 1], F32, tag="rowmax")
            nc.vector.tensor_max(rowmax[:, :], rm[:, 0:1], rm[:, 1:2])
            dfree_ps = epi_psum([32, 2], "dfree_ps")
            nc.tensor.matmul(dfree_ps[:, :], lhsT=posr[0][:, :],
                             rhs=dcol4[:, 0:2], start=True, stop=False)
            nc.tensor.matmul(dfree_ps[:, :], lhsT=posr[1][:, :],
                             rhs=dcol4[:, 2:4], start=False, stop=True)
            dfree_sb = small.tile([32, 1], F32, tag="dfree_sb")
            nc.scalar.copy(dfree_sb[:, :], dfree_ps[:, 0:1])
            rh_pre = small.tile([32, 2], F32, tag="rh_pre")
            nc.gpsimd.memset(rh_pre[:, :], 0.0)
            nc.vector.tensor_sub(rh_pre[:, 0:1], rowmax[:, :], dfree_sb[:, :])
            rowhinge = small.tile([32, 2], F32R, tag="rowhinge")
            nc.scalar.activation(rowhinge[:, :], rh_pre[:, :], ACT.Relu,
                                 bias=margin128[0:32, :])
            for h in range(2):
                rh_ps = epi_psum([128, 2], "rh_ps")
                nc.tensor.matmul(rh_ps[:, :], lhsT=postr[h][:, :],
                                 rhs=rowhinge[:, :], start=True, stop=True)
                nc.scalar.copy(payload[:, 4 + h:5 + h], rh_ps[:, 0:1])

            # payload -> DRAM (one DMA, contiguous per partition) -> AllGather
            if DEBUG:
                nc.sync.dma_start(out=dbgpay_e[:, :], in_=payload[:, :])
            nc.sync.dma_start(out=pay_d[:, :], in_=payload[:, :])
            nc.gpsimd.collective_compute(
                "AllGather", ALU.bypass,
                replica_groups=[list(range(NCORES))],
                ins=[pay_d.opt()],
                outs=[ag2_d.opt()],
            )

            # final combine (identical on every core): strided reload puts
            # sentences on partitions, (stat, core) along free; DVE reduces
            agg_in = small.tile([128, 6 * NCORES], F32, tag="agg_in")
            nc.sync.dma_start(
                out=agg_in.rearrange("p (c m) -> p c m", m=NCORES),
                in_=ag2_d.rearrange("(m p) c -> p c m", m=NCORES),
            )
            agv = agg_in.rearrange("p (c m) -> p c m", m=NCORES)
            agg = small.tile([128, 6], F32, tag="agg")
            nc.vector.tensor_reduce(out=agg[:, 0:2], in_=agv[:, 0:2, :],
                                    axis=AX.X, op=ALU.max)
            nc.vector.tensor_reduce(out=agg[:, 2:6], in_=agv[:, 2:6, :],
                                    axis=AX.X, op=ALU.add)
            ch2 = small.tile([128, 2], F32, tag="ch2")
            nc.vector.tensor_sub(ch2[:, :], agg[:, 0:2], agg[:, 2:4])
            hing4 = small.tile([128, 4], F32, tag="hing4")
            nc.scalar.activation(hing4[:, 0:2], ch2[:, :], ACT.Relu,
                                 bias=margin128[:, :])
            nc.scalar.copy(hing4[:, 2:4], agg[:, 4:6])
            psum128 = small.tile([128, 1], F32, tag="psum128")
            nc.vector.tensor_reduce(out=psum128[:, :], in_=hing4[:, :],
                                    axis=AX.X, op=ALU.add)
            lsum_ps = epi_psum([1, 128], "lsum_ps")
            nc.tensor.transpose(lsum_ps[:, :], psum128[:, :], ident[:, :])
            loss = small.tile([1, 1], F32, tag="loss")
            nc.vector.tensor_reduce(out=loss[:, :], in_=lsum_ps[:, :], axis=AX.X,
                                    op=ALU.add)
            nc.sync.dma_start(out=out_e[:, :], in_=loss[:, :])

    nc.finalize()
    return nc


# ---------------------------------------------------------------------------
# host side
# ---------------------------------------------------------------------------

def build_in_maps(plan, im_set, s_seq):
    im_set = np.asarray(im_set, dtype=np.float32)
    s_seq = np.asarray(s_seq, dtype=np.float32)
    NT, NT0, BIc = plan["NT"], plan["NT0"], plan["BIc"]
    cj = plan["cj_rows"]
    CJc = NT * 128

    s_rows = np.zeros((CJc, D), np.float32)
    idx = [i for i, row in enumerate(cj) if row is not None]
    cs = np.array([cj[i][0] for i in idx])
    js = np.array([cj[i][1] for i in idx])
    s_rows[idx] = s_seq[cs, js]
    st_f32 = np.ascontiguousarray(
        s_rows.reshape(NT, 128, KC, 128).transpose(0, 3, 2, 1))
    if MM_F8:
        st = np.clip(st_f32 * S_UPS, -240.0, 240.0).astype(F8)
    else:
        st = st_f32.astype(BF)
    srow = s_rows.reshape(NT, 128, D).astype(BF)

    gmat = np.zeros((128, NT * 128), BF)
    for t in range(NT):
        h = 0 if t < NT0 else 1
        for p in range(128):
            row = cj[128 * t + p]
            if row is not None:
                gmat[p, 128 * t + (row[0] - 128 * h)] = 1.0
    ident = np.eye(128, dtype=np.float32)

    in_maps = []
    for m in range(NCORES):
        imr = np.zeros((BIc, D), BF)
        pos0 = np.zeros((128, 32), np.float32)
        pos1 = np.zeros((128, 32), np.float32)
        for r in range(32):
            g = int(plan["assign"][r, m])
            l = int(plan["im_l"][g])
            o = int(plan["slot_off"][r])
            imr[o:o + l] = im_set[g, 1:1 + l]
            (pos0 if g < 128 else pos1)[g % 128, r] = 1.0
        in_maps.append({
            "imr": imr,
            "srow": srow,
            "st": st,
            "ident": ident,
            "gmat": gmat,
            "pos0": pos0,
            "pos1": pos1,
            "post0": np.ascontiguousarray(pos0.T),
            "post1": np.ascontiguousarray(pos1.T),
        })
    return in_maps


_NC_CACHE = {}


def kernel(im_set, s_seq, im_len, s_len):
    global LAST_RESULT
    plan = make_plan(im_len, s_len)
    key = plan["sig"]
    nc = _NC_CACHE.get(key)
    in_maps = build_in_maps(plan, im_set, s_seq)
    if nc is None:
        nc = build_nc(plan)
        _NC_CACHE[key] = nc
        # first executions of a fresh NEFF carry upload/launch-skew cost
        # (cores desynchronize by >100us); warm it up
        for _ in range(3):
            run_bass_kernel_spmd(nc, in_maps, core_ids=list(range(NCORES)))
    res = run_bass_kernel_spmd(nc, in_maps, core_ids=list(range(NCORES)))
    LAST_RESULT = res
    out = np.asarray(res.results[0]["out"], dtype=np.float32).reshape(())
    return out
